# revision 23
# baseline (speedup 1.0000x reference)
"""Trainium2 Bass kernel for a pre-LN transformer block (B=4, T=2048, E=1024, H=16).

Sharding: 8 cores = 4 batches x 2 token-halves. Each core receives its batch's
full 2048 tokens (rolled so its own 1024 query tokens come first), computes
K/V for all 2048 tokens (redundantly with its pair core) and everything else
(Q, attention, proj, FFN) only for its own 1024 tokens. Zero cross-core
communication; host reassembles the output.

v3 layout, organized around the softmax-exp wall on ACT (~260us of [128,1024]
Exp calls, the largest irreducible engine cost):
  - Emission order Q -> K(mf0) -> scores/exp(h0) -> V -> AV(h0) -> h1 ->
    {K(mf), att(2mf), att(2mf+1)}: the exp stream starts right after K's
    first column; V and the remaining K columns execute under the wall.
  - All attention-phase PSUM pools coexist (scores 2x2 banks, qkv 2x1,
    po 1x2) so no pool handoff gates the wall.
  - Q/K bias rides the DVE PSUM->SBUF evacuation (tensor_scalar with a
    per-partition AP scalar); ACT stays exp-only during attention.
  - AV contracts key-tile pairs with fp8 DoubleRow (exp emits fp8 at).
  - po is copied to SBUF right at head end (frees the single PSUM po slot),
    the normalize chain runs off-critical from the SBUF copy.
  - FFN is bf16 for precision, but W2 is resident fp8-E3M4 (x64 scale,
    4 mantissa bits keep max-err ~3x under the gate) so FFN2 streams no
    weights; the residual runs in a x64 domain (LN2 eps scaled by 64^2,
    one final descale on ACT).
  - LN transposes issue on the scalar queue (idle then), weights on gpsimd,
    x/xq/w2/outputs on sync: no DMA-xbar thrash on the copy queues.

LayerNorm gains/biases are folded into the matmul weights host-side.
"""

import numpy as np
import ml_dtypes

BF = ml_dtypes.bfloat16
F8 = ml_dtypes.float8_e4m3
E3 = ml_dtypes.float8_e3m4

B, T, E, H, HS, FF = 4, 2048, 1024, 16, 64, 4096
TQ = T // 2          # own query tokens per core
NCORES = 8
EPS = 1e-5
SCL = 64.0           # fp8 weight / residual-domain scale
NMT = T // 128       # 16 token tiles (full batch)
NMQ = TQ // 128      # 8 token tiles (own half)
NJE = E // 128       # 8 feature tiles of E
NJF = FF // 128      # 32 feature tiles of FF

_CACHE = {}
TRACE = False        # set by test harness to capture an NTFF profile
LAST_RESULTS = None  # BassKernelResults from the most recent run


def _build():
    import concourse.bacc as bacc
    import concourse.tile as tile
    from concourse import mybir
    from contextlib import ExitStack

    f32 = mybir.dt.float32
    bf16 = mybir.dt.bfloat16
    f8 = mybir.dt.float8e4
    f8e3 = mybir.dt.float8e3
    DR = mybir.MatmulPerfMode.DoubleRow
    AF = mybir.ActivationFunctionType
    OP = mybir.AluOpType

    nc = bacc.Bacc("TRN2", target_bir_lowering=False, debug=False,
                   num_devices=NCORES)

    # ---- DRAM I/O ----
    x_d = nc.declare_dram_parameter("x", [T, E], bf16, isOutput=False)
    wq_d = nc.declare_dram_parameter("wq", [NJE, 128, E], f8, isOutput=False)
    wk_d = nc.declare_dram_parameter("wk", [NJE, 128, E], f8, isOutput=False)
    wv_d = nc.declare_dram_parameter("wv", [NJE, 128, E], f8, isOutput=False)
    wo_d = nc.declare_dram_parameter("wo", [NJE, 128, E], f8, isOutput=False)
    w1_d = nc.declare_dram_parameter("w1", [NJF, 128, E], bf16, isOutput=False)
    w2_d = nc.declare_dram_parameter("w2", [NJF, 128, E], f8e3, isOutput=False)
    cq_d = nc.declare_dram_parameter("cq", [128, NJE], f32, isOutput=False)
    ck_d = nc.declare_dram_parameter("ck", [128, NJE], f32, isOutput=False)
    cvb_d = nc.declare_dram_parameter("cvb", [128, E], f32, isOutput=False)
    xq_d = nc.declare_dram_parameter("xq", [TQ, E], f32, isOutput=False)
    b2r_d = nc.declare_dram_parameter("b2r", [1, E], bf16, isOutput=False)
    b1c_d = nc.declare_dram_parameter("b1c", [128, NJF], f32, isOutput=False)
    out_d = nc.declare_dram_parameter("out", [TQ, E], f32, isOutput=True)
    rbounce = nc.dram_tensor("rbounce", [H, TQ], f32)

    def layernorm(stats_pool, x_sb, out_bf, eps_sb):
        st = stats_pool.tile([128, 2, 6], f32, name="ln_st")
        nc.vector.bn_stats(out=st[:, 0, :], in_=x_sb[:, 0:512])
        nc.vector.bn_stats(out=st[:, 1, :], in_=x_sb[:, 512:1024])
        mv = stats_pool.tile([128, 2], f32, name="ln_mv")
        nc.vector.bn_aggr(out=mv[:], in_=st[:])
        rstd = stats_pool.tile([128, 1], f32, name="ln_rstd")
        nc.scalar.activation(out=rstd[:], in_=mv[:, 1:2], func=AF.Sqrt,
                             bias=eps_sb[:])
        nc.vector.reciprocal(out=rstd[:], in_=rstd[:])
        nmr = stats_pool.tile([128, 1], f32, name="ln_nmr")
        nc.vector.tensor_tensor(out=nmr[:], in0=mv[:, 0:1], in1=rstd[:],
                                op=OP.mult)
        nc.vector.tensor_scalar_mul(out=nmr[:], in0=nmr[:], scalar1=-1.0)
        nc.scalar.activation(out=out_bf[:], in_=x_sb[:], func=AF.Identity,
                             bias=nmr[:], scale=rstd[:])

    with tile.TileContext(nc) as tc:
        top = ExitStack()
        const = top.enter_context(tc.tile_pool(name="const", bufs=1, side="left"))
        eps_sb = const.tile([128, 1], f32)
        nc.vector.memset(eps_sb[:], EPS)
        eps2_sb = const.tile([128, 1], f32)
        nc.vector.memset(eps2_sb[:], EPS * SCL * SCL)
        cq_sb = const.tile([128, NJE], f32)
        nc.sync.dma_start(out=cq_sb[:], in_=cq_d[:])
        ck_sb = const.tile([128, NJE], f32)
        nc.sync.dma_start(out=ck_sb[:], in_=ck_d[:])
        cv_sb = const.tile([128, E], f32)
        nc.sync.dma_start(out=cv_sb[:], in_=cvb_d[:])
        ones_sb = const.tile([1, 512], bf16)
        nc.vector.memset(ones_sb[:], 1.0)
        b2r_sb = const.tile([1, E], bf16)
        nc.sync.dma_start(out=b2r_sb[:], in_=b2r_d[:])

        # persistent QKV-era activations (right stack, closed after attention)
        qkv_es = ExitStack()
        qkv_pool = qkv_es.enter_context(tc.tile_pool(name="qkvact", bufs=1, side="right"))
        # q^T zero-padded per head: head h occupies partitions (h%2)*64..+64
        # of slice [:, h, :]; the scores matmul contracts the full K=128
        # (keeps PE HAM at full clock).
        qT = qkv_pool.tile([128, H, TQ], bf16)
        kT = qkv_pool.tile([128, NJE, T], bf16)
        v_aug = qkv_pool.tile([128, NMT // 2, 2, H, HS + 1], f8)
        h8 = [qkv_pool.tile([128, NJE, 4, 128], f8, name=f"h8{g}")
              for g in range(4)]

        # ---------- Phase 1: LN1 + transpose ----------
        hG_es = ExitStack()
        hGp = hG_es.enter_context(tc.tile_pool(name="hG", bufs=1, side="left"))
        hG = [hGp.tile([128, 4, NJE, 128], bf16, name=f"hG{g}")
              for g in range(4)]
        with tc.tile_pool(name="ln1", bufs=16, side="left") as xin, \
             tc.tile_pool(name="ln1s", bufs=10, side="left") as stp, \
             tc.tile_pool(name="ln1h", bufs=5, side="left") as hbp:
            xt = []
            for mt in range(NMT):
                x_sb = xin.tile([128, E], bf16)
                nc.sync.dma_start(out=x_sb[:], in_=x_d[mt * 128:(mt + 1) * 128, :])
                xt.append(x_sb)
            for mt in range(NMT):
                h_bf = hbp.tile([128, E], bf16)
                layernorm(stp, xt[mt], h_bf, eps_sb)
                nc.scalar.dma_start_transpose(out=hG[mt // 4][:, mt % 4, :, :],
                                              in_=h_bf[:])
                nc.vector.tensor_copy(out=h8[mt // 4][:, :, mt % 4, :],
                                      in_=hG[mt // 4][:, mt % 4, :, :])
        hG_es.close()

        # left-stack pools that outlive attention (opened in LIFO order:
        # h2G until FFN1 end, wo/oT until proj end)
        h2T_es = ExitStack()
        h2G = h2T_es.enter_context(tc.tile_pool(name="h2T", bufs=1, side="left")) \
            .tile([128, NMQ, NJE, 128], bf16)
        wop_es = ExitStack()
        wo_sb = wop_es.enter_context(tc.tile_pool(name="proj_w", bufs=1, side="left")) \
            .tile([128, NJE, E], f8)
        for j in range(NJE):
            nc.sync.dma_start(out=wo_sb[:, j, :], in_=wo_d[j])
        oT_es = ExitStack()
        oT = oT_es.enter_context(tc.tile_pool(name="oT", bufs=1, side="left")) \
            .tile([128, NJE, TQ], f8)                  # xSCL normalized attn out^T

        # ---------- Phase 2+3: QKV + attention, fused ----------
        with tc.tile_pool(name="att_ps", bufs=2, space="PSUM") as aps, \
             tc.tile_pool(name="qkv_ps", bufs=2, space="PSUM") as qkps, \
             tc.tile_pool(name="att_po", bufs=1, space="PSUM") as ops, \
             tc.tile_pool(name="w_pool", bufs=2, side="right") as wqkv, \
             tc.tile_pool(name="att_t", bufs=6, side="right") as atp, \
             tc.tile_pool(name="att_r", bufs=2, side="right") as rp, \
             tc.tile_pool(name="att_rb", bufs=2, side="right") as rbp, \
             tc.tile_pool(name="att_pos", bufs=2, side="right") as pop:

            wq_sb = wqkv.tile([128, NJE, E], f8, name="wt")
            for j in range(NJE):
                nc.gpsimd.dma_start(out=wq_sb[:, j, :], in_=wq_d[j])
            wk_sb = wqkv.tile([128, NJE, E], f8, name="wt")
            for j in range(NJE):
                nc.gpsimd.dma_start(out=wk_sb[:, j, :], in_=wk_d[j])
            for h in range(H):
                p0 = 64 - (h % 2) * 64  # zero the OTHER head's q rows
                nc.gpsimd.memset(qT[p0:p0 + 64, h, :], 0.0)
            nc.gpsimd.memset(v_aug[:, :, :, :, HS:HS + 1], 1.0 / SCL)

            # --- Q (all heads; head pair 2mf,2mf+1 finishes per mf) ---
            for mf in range(NJE):
                for g in range(2):
                    pq = qkps.tile([128, 512], f32, name="ps_qkv")
                    for j in range(0, NJE, 2):
                        lhsT = wq_sb[:, j:j + 2, mf * 128:(mf + 1) * 128]
                        rhs = h8[g][:, j:j + 2, :, :]
                        nc.tensor.matmul(pq[:], lhsT, rhs, perf_mode=DR,
                                         start=(j == 0), stop=(j == NJE - 2))
                    h0, h1 = 2 * mf, 2 * mf + 1
                    sl = slice(g * 512, (g + 1) * 512)
                    nc.vector.tensor_scalar_add(
                        out=qT[0:64, h0, sl], in0=pq[0:64, :],
                        scalar1=cq_sb[0:64, mf:mf + 1])
                    nc.vector.tensor_scalar_add(
                        out=qT[64:128, h1, sl], in0=pq[64:128, :],
                        scalar1=cq_sb[64:128, mf:mf + 1])

            def emit_k(mf):
                for g in range(4):
                    pk = qkps.tile([128, 512], f32, name="ps_qkv")
                    for j in range(0, NJE, 2):
                        lhsT = wk_sb[:, j:j + 2, mf * 128:(mf + 1) * 128]
                        rhs = h8[g][:, j:j + 2, :, :]
                        nc.tensor.matmul(pk[:], lhsT, rhs, perf_mode=DR,
                                         start=(j == 0), stop=(j == NJE - 2))
                    nc.vector.tensor_scalar_add(
                        out=kT[:, mf, g * 512:(g + 1) * 512], in0=pk[:],
                        scalar1=ck_sb[:, mf:mf + 1])

            at_by_pair = {}
            po_by_head = {}

            def emit_scores_exp(h):
                for st in range(NMT):
                    ps = aps.tile([128, TQ], f32, name="ps_sc")
                    lhsT = kT[:, h // 2, st * 128:(st + 1) * 128]
                    nc.tensor.matmul(ps[:, 0:512], lhsT, qT[:, h, 0:512],
                                     start=True, stop=True)
                    nc.tensor.matmul(ps[:, 512:1024], lhsT, qT[:, h, 512:1024],
                                     start=True, stop=True)
                    if st % 2 == 0:
                        at_by_pair[(h, st // 2)] = atp.tile([128, 2, TQ], f8,
                                                            name="att")
                    at2 = at_by_pair[(h, st // 2)]
                    nc.scalar.activation(out=at2[:, st % 2, :], in_=ps[:],
                                         func=AF.Exp, scale=float(HS) ** -0.5)

            def emit_av_finish(h):
                po = ops.tile([HS + 1, TQ], f32, name="ps_o")
                for sp in range(NMT // 2):
                    at2 = at_by_pair.pop((h, sp))
                    vk = v_aug[:, sp, :, h, :]          # [128, 2, HS+1] fp8
                    nc.tensor.matmul(po[:, 0:512], vk, at2[:, :, 0:512],
                                     perf_mode=DR,
                                     start=(sp == 0), stop=(sp == NMT // 2 - 1))
                    nc.tensor.matmul(po[:, 512:1024], vk, at2[:, :, 512:1024],
                                     perf_mode=DR,
                                     start=(sp == 0), stop=(sp == NMT // 2 - 1))
                # copy po to SBUF immediately: frees the single PSUM po slot,
                # the normalize chain below runs off-critical from the copy
                posb = pop.tile([HS + 1, TQ], f32, name="posb")
                nc.vector.tensor_copy(out=posb[:], in_=po[:])
                # v_aug ones column is 1/SCL so r1 = SCL/denom and
                # oT = SCL * softmax(scores) @ v  (fp8-friendly magnitude)
                r1 = rp.tile([1, TQ], f32, name="rsum")
                nc.vector.reciprocal(out=r1[:], in_=posb[HS:HS + 1, :])
                nc.sync.dma_start(out=rbounce[h:h + 1, :], in_=r1[:])
                rb = rbp.tile([64, TQ], f32, name="rbc")
                nc.sync.dma_start(out=rb[:],
                                   in_=rbounce[h:h + 1, :].to_broadcast([64, TQ]))
                p0 = (h % 2) * 64
                nc.vector.tensor_tensor(out=oT[p0:p0 + 64, h // 2, :],
                                        in0=posb[0:HS, :], in1=rb[:], op=OP.mult)

            # --- K(0), scores/exp(h0) first; V slides under h0's exp wall
            # (AV(h0) is emitted after V so the at-pair slots pace exp until
            # v_aug catches up) ---
            def emit_v(st):
                pv0 = qkps.tile([128, 512], f32, name="ps_qkv")
                pv1 = qkps.tile([128, 512], f32, name="ps_qkv")
                for j in range(0, NJE, 2):
                    lhsT = h8[st // 4][:, j:j + 2, st % 4, :]
                    nc.tensor.matmul(pv0[:], lhsT,
                                     wv_sb[:, j:j + 2, 0:512], perf_mode=DR,
                                     start=(j == 0), stop=(j == NJE - 2))
                    nc.tensor.matmul(pv1[:], lhsT,
                                     wv_sb[:, j:j + 2, 512:1024], perf_mode=DR,
                                     start=(j == 0), stop=(j == NJE - 2))
                sp, o = st // 2, st % 2
                nc.vector.tensor_tensor(
                    out=v_aug[:, sp, o, 0:8, 0:HS],
                    in0=pv0.rearrange("p (h d) -> p h d", h=8),
                    in1=cv_sb[:, 0:512].rearrange("p (h d) -> p h d", h=8),
                    op=OP.add)
                nc.vector.tensor_tensor(
                    out=v_aug[:, sp, o, 8:16, 0:HS],
                    in0=pv1.rearrange("p (h d) -> p h d", h=8),
                    in1=cv_sb[:, 512:1024].rearrange("p (h d) -> p h d", h=8),
                    op=OP.add)

            emit_k(0)
            emit_scores_exp(0)
            wv_sb = wqkv.tile([128, NJE, E], f8, name="wt")
            for j in range(NJE):
                nc.gpsimd.dma_start(out=wv_sb[:, j, :], in_=wv_d[j])
            for st in range(NMT // 2):
                emit_v(st)
            emit_scores_exp(1)
            for st in range(NMT // 2, NMT):
                emit_v(st)
            emit_av_finish(0)
            emit_av_finish(1)
            for mf in range(1, NJE):
                emit_k(mf)
                for h in (2 * mf, 2 * mf + 1):
                    emit_scores_exp(h)
                    emit_av_finish(h)

        qkv_es.close()

        # ---------- Phase 4+5: attn projection + residual (xSCL) + LN2 ----------
        xr_pool = top.enter_context(tc.tile_pool(name="xr", bufs=1, side="right"))
        xr_t = [xr_pool.tile([128, E], f32, name=f"xr{i}") for i in range(NMQ)]
        ffnT = top.enter_context(tc.tile_pool(name="ffnT", bufs=1, side="right")) \
            .tile([128, NJF, TQ], bf16)
        # prefetch FFN1 weights on the right stack: these tiles come from the
        # attention pools' freed region, so the DMAs run during proj instead
        # of being address-gated on the proj pools' release
        w1pre_pool = top.enter_context(tc.tile_pool(name="w1pre", bufs=1, side="right"))
        b1_sb = w1pre_pool.tile([128, NJF], f32)
        nc.gpsimd.dma_start(out=b1_sb[:], in_=b1c_d[:])
        w1pre = []
        for mf in range(4):
            t = w1pre_pool.tile([128, NJE, 128], bf16, name=f"w1p{mf}")
            nc.gpsimd.dma_start(out=t[:],
                                in_=w1_d[mf].rearrange("p (j c) -> p j c", j=NJE))
            w1pre.append(t)
        f1_es = ExitStack()
        f1ps = f1_es.enter_context(tc.tile_pool(name="f1ps", bufs=3, space="PSUM"))
        f1wp = f1_es.enter_context(tc.tile_pool(name="f1w", bufs=4, side="right"))
        with tc.tile_pool(name="proj_ps", bufs=4, space="PSUM") as pps, \
             tc.tile_pool(name="ln2s", bufs=10, side="left") as stp2, \
             tc.tile_pool(name="ln2h", bufs=5, side="left") as hbp2, \
             tc.tile_pool(name="xq_jit", bufs=2, side="left") as xqp:
            for mt in range(NMQ):
                x_sb = xqp.tile([128, E], f32, name="xq")
                nc.sync.dma_start(out=x_sb[:], in_=xq_d[mt * 128:(mt + 1) * 128, :])
                pa = pps.tile([128, 512], f32, name="ps_pr")
                pb = pps.tile([128, 512], f32, name="ps_pr")
                for j in range(0, NJE, 2):
                    lhsT = oT[:, j:j + 2, mt * 128:(mt + 1) * 128]
                    nc.tensor.matmul(pa[:], lhsT,
                                     wo_sb[:, j:j + 2, 0:512], perf_mode=DR,
                                     start=(j == 0), stop=(j == NJE - 2))
                    nc.tensor.matmul(pb[:], lhsT,
                                     wo_sb[:, j:j + 2, 512:1024], perf_mode=DR,
                                     start=(j == 0), stop=(j == NJE - 2))
                # xr = SCL*(x + bo) + SCL*(o @ Wo)   [x64 domain]
                nc.vector.tensor_tensor(out=xr_t[mt][:, 0:512], in0=pa[:],
                                        in1=x_sb[:, 0:512], op=OP.add)
                nc.vector.tensor_tensor(out=xr_t[mt][:, 512:1024], in0=pb[:],
                                        in1=x_sb[:, 512:1024], op=OP.add)
                h_bf = hbp2.tile([128, E], bf16)
                layernorm(stp2, xr_t[mt][:], h_bf, eps2_sb)
                nc.scalar.dma_start_transpose(out=h2G[:, mt, :, :], in_=h_bf[:])

            # ---------- Phase 6: FFN1 (bf16), overlapping proj tail ----------
            for mf in range(NJF):
                if mf < 4:
                    w1_sb = w1pre[mf]
                else:
                    w1_sb = f1wp.tile([128, NJE, 128], bf16, name="w1t")
                    nc.gpsimd.dma_start(out=w1_sb[:],
                                        in_=w1_d[mf].rearrange("p (j c) -> p j c", j=NJE))
                for g in range(2):
                    pf = f1ps.tile([128, 512], f32, name="ps_f1")
                    for j in range(NJE):
                        nc.tensor.matmul(pf[:], w1_sb[:, j, :],
                                         h2G[:, g * 4:(g + 1) * 4, j, :],
                                         start=(j == 0), stop=(j == NJE - 1))
                    nc.scalar.activation(out=ffnT[:, mf, g * 512:(g + 1) * 512],
                                         in_=pf[:], func=AF.Relu,
                                         bias=b1_sb[:, mf:mf + 1])

        oT_es.close()
        wop_es.close()

        # ---------- Phase 7: FFN2 (bf16 x resident E3M4 W2) ----------
        w2_es = ExitStack()
        w2res = w2_es.enter_context(tc.tile_pool(name="w2res", bufs=1, side="left")) \
            .tile([128, NJF, E], f8e3)
        for k in range(NJF):
            nc.sync.dma_start(out=w2res[:, k, :], in_=w2_d[k])
        if True:
            with tc.tile_pool(name="f2o", bufs=4, side="left") as f2op, \
                 tc.tile_pool(name="f2ps", bufs=5, space="PSUM") as f2ps:
                for nbh in range(2):
                    for tg in range(2):
                        mts = range(tg * 4, tg * 4 + 4)
                        psums = {mt: f2ps.tile([128, 512], f32, name="ps_f2")
                                 for mt in mts}
                        for mt in mts:
                            nc.tensor.matmul(
                                psums[mt][:], ones_sb[0:1, 0:128],
                                b2r_sb[0:1, nbh * 512:(nbh + 1) * 512],
                                start=True, stop=False, skip_group_check=True)
                        for k in range(NJF):
                            for mt in mts:
                                nc.tensor.matmul(
                                    psums[mt][:],
                                    ffnT[:, k, mt * 128:(mt + 1) * 128],
                                    w2res[:, k, nbh * 512:(nbh + 1) * 512],
                                    start=False, stop=(k == NJF - 1),
                                    skip_group_check=True)
                        for mt in mts:
                            o_sb = f2op.tile([128, 512], f32, name="osb")
                            nc.vector.tensor_tensor(
                                out=o_sb[:], in0=psums[mt][:],
                                in1=xr_t[mt][:, nbh * 512:(nbh + 1) * 512],
                                op=OP.add)
                            out_sb = f2op.tile([128, 512], f32, name="outsb")
                            nc.scalar.activation(out=out_sb[:], in_=o_sb[:],
                                                 func=AF.Identity,
                                                 scale=1.0 / SCL)
                            nc.sync.dma_start(
                                out=out_d[mt * 128:(mt + 1) * 128,
                                          nbh * 512:(nbh + 1) * 512],
                                in_=out_sb[:])

        w2_es.close()
        f1_es.close()
        h2T_es.close()
        top.close()

    nc.compile()
    return nc


def _prep_weights(ln1_g, ln1_b, Wq, Wk, Wv, Wo, bo, ln2_g, ln2_b, W1, b1, W2, b2):
    f64 = np.float64
    g1 = np.asarray(ln1_g, f64)
    b1ln = np.asarray(ln1_b, f64)
    g2 = np.asarray(ln2_g, f64)
    b2ln = np.asarray(ln2_b, f64)

    def flat_qkv(W):
        return np.asarray(W, f64).transpose(1, 0, 2).reshape(E, H * HS)

    Wqf, Wkf, Wvf = flat_qkv(Wq), flat_qkv(Wk), flat_qkv(Wv)
    out = {}
    out["wq"] = np.ascontiguousarray((g1[:, None] * Wqf).reshape(NJE, 128, E).astype(F8))
    out["wk"] = np.ascontiguousarray((g1[:, None] * Wkf).reshape(NJE, 128, E).astype(F8))
    out["wv"] = np.ascontiguousarray((g1[:, None] * Wvf).reshape(NJE, 128, E).astype(F8))
    cq = (b1ln @ Wqf).astype(np.float32)
    ck = (b1ln @ Wkf).astype(np.float32)
    cv = (b1ln @ Wvf).astype(np.float32)
    out["cq"] = np.ascontiguousarray(cq.reshape(NJE, 128).T)
    out["ck"] = np.ascontiguousarray(ck.reshape(NJE, 128).T)
    out["cvb"] = np.ascontiguousarray(np.broadcast_to(cv, (128, E)))
    out["wo"] = np.ascontiguousarray(np.asarray(Wo, f64).reshape(NJE, 128, E).astype(F8))
    W1p = g2[:, None] * np.asarray(W1, f64)
    b1p = (np.asarray(b1, f64) + b2ln @ np.asarray(W1, f64)).astype(np.float32)
    out["w1"] = np.ascontiguousarray(
        W1p.reshape(NJE, 128, NJF, 128).transpose(2, 1, 0, 3).reshape(NJF, 128, E).astype(BF))
    out["b1c"] = np.ascontiguousarray(b1p.reshape(NJF, 128).T)
    out["w2"] = np.ascontiguousarray(
        (SCL * np.asarray(W2, f64)).reshape(NJF, 128, E).astype(E3))
    out["b2r"] = np.ascontiguousarray(
        (SCL * np.asarray(b2, f64)).astype(BF).reshape(1, E))
    return out


def kernel(x, ln1_g, ln1_b, Wq, Wk, Wv, Wo, bo, ln2_g, ln2_b, W1, b1, W2, b2):
    global LAST_RESULTS
    from concourse.bass_utils import run_bass_kernel_spmd

    if "nc" not in _CACHE:
        _CACHE["nc"] = _build()
    nc = _CACHE["nc"]

    wmap = _prep_weights(ln1_g, ln1_b, Wq, Wk, Wv, Wo, bo,
                         ln2_g, ln2_b, W1, b1, W2, b2)
    x = np.asarray(x, np.float32)

    in_maps = []
    for c in range(NCORES):
        b, half = c // 2, c % 2
        xb = x[b]
        x_roll = np.ascontiguousarray(
            np.concatenate([xb[half * TQ:], xb[:half * TQ]], axis=0))
        m = dict(wmap)
        m["x"] = x_roll.astype(BF)
        m["xq"] = np.ascontiguousarray(
            SCL * (x_roll[:TQ] + np.asarray(bo, np.float32)[None, :]))
        in_maps.append(m)

    res = run_bass_kernel_spmd(nc, in_maps, list(range(NCORES)), trace=TRACE)
    LAST_RESULTS = res

    out = np.empty((B, T, E), np.float32)
    for c in range(NCORES):
        b, half = c // 2, c % 2
        out[b, half * TQ:(half + 1) * TQ] = res.results[c]["out"]
    return out
